# revision 69
# baseline (speedup 1.0000x reference)
"""Complex-valued multi-head attention on 8 Trainium2 NeuronCores.

Sharding: batch(2) x head-pairs(4) -> 8 cores; each core runs one batch
element and 2 heads end-to-end (QKV proj -> complex scores -> |s| softmax
-> AV -> partial W_O), host sums the W_O partials over the 4 cores of each
batch element (tensor-parallel reduce) and transposes to the output layout.

Restructure vs the 257456ns baseline (engine-balance rework, 236107ns
cost-model):
- score extraction: sq = re^2|im^2 either as ONE fused ACT Square
  [128,1024] (psum read, unary -- Square lives in every act table so it
  never forces a table load), or as DVE psum->f16 copy + ONE Pool sbuf
  square [128,1024]; z-adds [128,512] on Pool (last 2 tiles per half on
  DVE so the final sqrt chunk isn't stuck behind the Pool queue).
- V is projected directly in transposed ([k-pos, dv]) layout via
  moving-side weights (wvpack rhs blocks), eliminating the 32 PE
  transposes and the second extraction pass; V(6),V(7) deferred into
  qc1's stream (their xt tiles are the last loads, so they stay live).
- softmax rowsums accumulate on PE (ones matmul into a psum row).
- sqrt runs as 2048-col chunks emitted inline with the score stream;
  exps split x8 and chained after them.  All table-based ACT ops carry
  explicit chain deps (add_dep_helper) so the scheduler cannot
  interleave Sqrt/Exp phases: 11 table loads total (vs 47 unpinned).
- AV(prev qc) is packed into score-stream slots (AV_SLOTS), delayed so
  the in-order PE queue never waits for an exp chunk, late k-tiles
  doubled up so nothing trails the stream.
- q-projections of n-chunks 2..7 deferred out of qc0 (small xtq
  re-DMAs) to shorten the PE-bound qc0; startup DMAs interleaved
  (xt thirds with the weight segment each projection actually needs).
- DMA ring parallelism: weight DMAs and the in-stream xt v-blocks are
  issued from the gpsimd (Pool) DGE ring while the x tensors stream on
  the sync ring, halving the serialized startup-DMA span.
- W_O partials written as f16 (host sums in f64; quantization ~5e-4 rel).
"""
import sys

sys.path.insert(0, "/opt/trn_rl_repo")

import numpy as np

B, NQ, NK, R = 2, 2048, 2048, 512
H, DK, DV = 8, 64, 64
NCORES = 8
NCC = 8          # n-chunks for projection streaming (2048/256)
NCW = 256        # projection n-chunk width
QC = 4           # q-chunks in attention (2048/512)
QCW = 512
KT = 16          # k-tiles (2048/128)
HKT = 8          # k-tiles per half-batch

# engine-mix tuning
# fused-square positions per half: in half A the ACT is busy with the
# previous chunk's exps until ~tile 11, so fused tiles sit late there.
ACT_FUSED_A = (11, 13)
ACT_FUSED_B = (1, 3, 5, 9, 11, 13)
ACT_FUSED_Q0 = range(0, 12)              # qc0 half A (no exps on ACT)
ACT_FUSED_Q0B = range(0, 12)             # qc0 half B (gates sqrt_B_c3)
ZADD_DVE_NUM = 0      # out of ZADD_MOD z-adds go to DVE (rest Pool)
ZADD_MOD = 8
ST_ACT_NUM = 0        # out of ST_MOD W_O output copies go to ACT
ST_MOD = 5
PRJ_ACT = False       # q/A projection copies on ACT
V16_DVE = True        # v16 extraction on DVE
AV_DELAY = 3          # k-tile pairs to delay AV behind the score stream
EXP_SPLIT = 8         # exp batch granularity (ops per q-chunk)
EXP_EARLY_HALF = False  # emit h0-half exps right after sqrt_A chunks
EXP_EARLY_QC0 = False  # qc0 early-half exps: delays sqrt_B, loses
PJQ_SLOT_A = 4        # half-A slot for the deferred q projection
QC3_EARLY_EXP = True  # qc3 h0-half exps between the halves
BC_ACT = False        # bc_sb copy on ACT
PJQ_ACT = False       # deferred-q projection copy on ACT
V_DEFER = True        # defer V(6),V(7) projections into qc1
LAST_POOL_RS = False  # final AV block: rowsums on Pool instead of PE
SQRT_CHUNKS = 4       # sqrt chunks per half-stream
BCSB_F16 = True       # bc_sb (1/rowsum broadcast) in f16
AV_SLOTS = {3: (0,), 4: (1,), 5: (2,), 6: (3, 4),
            7: (5,), 8: (6, 7), 9: (8,), 10: (9, 10),
            11: (11,), 12: (12,), 13: (13,), 14: (14,), 15: (15,)}
AV_SLOTS_QC1 = AV_SLOTS

_CACHE = {}


def _build_nc():
    import concourse.bass as bass
    import concourse.tile as tile
    from concourse.tile import add_dep_helper
    from concourse import bacc, mybir

    f32 = mybir.dt.float32
    f32r = mybir.dt.float32r
    f16 = mybir.dt.float16
    ALU = mybir.AluOpType
    AF = mybir.ActivationFunctionType
    BCSB_DT = f16 if BCSB_F16 else f32

    nc = bacc.Bacc("TRN2", target_bir_lowering=False, debug=False,
                   num_devices=NCORES)

    xpack_e = nc.dram_tensor("xpack", [NCC, 24, 128, NCW], f16,
                             kind="ExternalInput")
    wpack_e = nc.dram_tensor("wpack", [128, 32 * 128], f16,
                             kind="ExternalInput")
    wvpack_e = nc.dram_tensor("wvpack", [128, 8 * 256], f16,
                              kind="ExternalInput")
    wopack_e = nc.dram_tensor("wopack", [128, 3 * 512], f16,
                              kind="ExternalInput")
    onesr_e = nc.dram_tensor("onesr", [1, 128], f32r, kind="ExternalInput")
    ore_e = nc.dram_tensor("out_re", [512, NQ], f16, kind="ExternalOutput")
    oim_e = nc.dram_tensor("out_im", [512, NQ], f16, kind="ExternalOutput")

    with tile.TileContext(nc) as tc:
      with nc.allow_low_precision(reason="fp16 softmax path"):
        with tc.tile_pool(name="pers", bufs=1) as pers, \
             tc.tile_pool(name="work", bufs=2) as work, \
             tc.tile_pool(name="psA", bufs=1, space="PSUM") as psA:

            # startup DMA order: interleave xt0 thirds with the weight
            # segments they unblock, so the first projection matmul can
            # start after ~1/3 of xt0 + wp seg0 instead of the full set.
            wp = pers.tile([128, 32 * 128], f16, tag="wp")
            wvp = pers.tile([128, 8 * 256], f16, tag="wvp")
            xt0_pre = work.tile([128, 24 * NCW], f16, tag="xt",
                                name="xt_0")

            def xt_part(xt, ncc, t0, t1):
                nc.sync.dma_start(
                    xt[:, t0 * 4 * NCW:t1 * 4 * NCW].rearrange(
                        "p (b f) -> p b f", f=NCW),
                    xpack_e[ncc, t0 * 4:t1 * 4].rearrange("b p f -> p b f"))

            xt1_pre0 = work.tile([128, 24 * NCW], f16, tag="xt",
                                 name="xt_1")
            xt_part(xt0_pre, 0, 0, 2)                       # q blocks
            nc.gpsimd.dma_start(wp[:, 0:2048], wpack_e[:, 0:2048])
            xt_part(xt0_pre, 0, 2, 4)                       # k blocks
            nc.gpsimd.dma_start(wp[:, 2048:4096], wpack_e[:, 2048:4096])
            xt_part(xt1_pre0, 1, 0, 2)
            xt_part(xt1_pre0, 1, 2, 4)
            nc.gpsimd.dma_start(wvp[:], wvpack_e[:])
            xt_part(xt0_pre, 0, 4, 6)                       # v blocks
            xt_part(xt1_pre0, 1, 4, 6)
            wop = pers.tile([128, 3 * 512], f16, tag="wop")
            nc.gpsimd.dma_start(wop[:], wopack_e[:])
            ones_row = pers.tile([1, 128], f32r, tag="ones_row")
            nc.gpsimd.dma_start(ones_row[:], onesr_e[:])
            ones16 = pers.tile([128, 1], f16, tag="ones16")
            nc.vector.memset(ones16[:], 1.0)
            eb_exp = pers.tile([128, 1], f32, tag="eb_exp")
            nc.vector.memset(eb_exp[:], -1.5)          # exp(mag - 1.5)

            # ---- projection destinations (h-major: cols h*2048 + n) ----
            q_all = pers.tile([128, 2 * NQ], f16, tag="q_all")
            q2_all = pers.tile([128, 2 * NQ], f16, tag="q2_all")
            A_all = pers.tile([128, 2 * NK], f16, tag="A_all")
            v16_all = pers.tile([128, 2 * NK], f16, tag="v16_all")
            oT_re = pers.tile([128, NQ], f16, tag="oT_re")
            oT_im = pers.tile([128, NQ], f16, tag="oT_im")

            grp_dest = [q_all, A_all]

            cnt = {"zadd": 0, "stc": 0}
            # ACT table-phase chaining: Sqrt and Exp live in different act
            # function tables (Square/Copy are in every table).  Without
            # explicit edges the scheduler interleaves sqrt chunks with
            # exp chunks and pays 1283ns per table switch.  Chaining all
            # table-based ACT ops in emission order keeps each table phase
            # contiguous (emission order already groups them).
            phase = {"last": None}

            def chain(ins):
                if phase["last"] is not None:
                    add_dep_helper(ins.ins, phase["last"].ins,
                                   reason="act table phase order")
                phase["last"] = ins
                return ins

            def xt_load(ncc):
                # k blocks first (A projection is the critical path) on
                # the sync ring, v blocks in parallel on the Pool ring.
                xt = work.tile([128, 24 * NCW], f16, tag="xt",
                               name=f"xt_{ncc}")
                xt_part(xt, ncc, 2, 4)
                nc.gpsimd.dma_start(
                    xt[:, 16 * NCW:24 * NCW].rearrange(
                        "p (b f) -> p b f", f=NCW),
                    xpack_e[ncc, 16:24].rearrange("b p f -> p b f"))
                return xt

            def wblk(w, rc):
                return wp[:, (w * 4 + rc) * 128:(w * 4 + rc + 1) * 128]

            def proj_q(ncc, xt, xoff=0):
                """q projection + q2 for n-chunk ncc; xoff=None marks the
                deferred path (q copy goes to ACT instead of DVE)."""
                def xblk(t, rc):
                    b = ((xoff or 0) + t * 4 + rc)
                    return xt[:, b * NCW:(b + 1) * NCW]

                cs0 = ncc * NCW
                pj = psA.tile([128, 512], f32, tag="sb", bufs=2,
                              name=f"pjq_{ncc}")
                for sub in range(2):
                    dst = pj[:, sub * 256:(sub + 1) * 256]
                    for rc in range(4):
                        nc.tensor.matmul(dst, wblk(2 * sub, rc),
                                         xblk(0, rc),
                                         start=(rc == 0), stop=False)
                    for rc in range(4):
                        nc.tensor.matmul(dst, wblk(2 * sub + 1, rc),
                                         xblk(1, rc),
                                         start=False, stop=(rc == 3))
                dap = q_all[:].rearrange("p (h n) -> p h n", h=2)[
                    :, :, cs0:cs0 + NCW]
                pap = pj[:].rearrange("p (h n) -> p h n", h=2)
                if PRJ_ACT or (xoff is None and PJQ_ACT):
                    nc.scalar.copy(dap, pap)
                else:
                    nc.vector.tensor_copy(dap, pap)
                # q2 = [q_i; -q_r] for this chunk (both heads) on DVE
                q2ap_t = q2_all[0:64].rearrange("p (h n) -> p h n", h=2)[
                    :, :, cs0:cs0 + NCW]
                qap_b = q_all[64:128].rearrange("p (h n) -> p h n", h=2)[
                    :, :, cs0:cs0 + NCW]
                nc.vector.tensor_scalar_mul(q2ap_t, qap_b, 1.0)
                q2ap_b = q2_all[64:128].rearrange("p (h n) -> p h n", h=2)[
                    :, :, cs0:cs0 + NCW]
                qap_t = q_all[0:64].rearrange("p (h n) -> p h n", h=2)[
                    :, :, cs0:cs0 + NCW]
                nc.vector.tensor_scalar_mul(q2ap_b, qap_t, -1.0)

            def proj_q_deferred(ncc):
                # late q projection: re-DMA just the q blocks of this
                # n-chunk (xt tiles from qc0 are long recycled)
                xtq = work.tile([128, 8 * NCW], f16, tag="xtq",
                                name=f"xtq_{ncc}")
                nc.sync.dma_start(
                    xtq[:].rearrange("p (b f) -> p b f", f=NCW),
                    xpack_e[ncc, 0:8].rearrange("b p f -> p b f"))
                proj_q(ncc, xtq, xoff=None)

            def proj_A(ncc, xt):
                def xblk(t, rc):
                    return xt[:, (t * 4 + rc) * NCW:(t * 4 + rc + 1) * NCW]

                cs0 = ncc * NCW
                # A (k) projection: stationary weights, [out, n]
                pj = psA.tile([128, 512], f32, tag="sb", bufs=2,
                              name=f"pjA_{ncc}")
                for sub in range(2):
                    s = 2 + sub
                    dst = pj[:, sub * 256:(sub + 1) * 256]
                    for rc in range(4):
                        nc.tensor.matmul(dst, wblk(2 * s, rc),
                                         xblk(2, rc),
                                         start=(rc == 0), stop=False)
                    for rc in range(4):
                        nc.tensor.matmul(dst, wblk(2 * s + 1, rc),
                                         xblk(3, rc),
                                         start=False, stop=(rc == 3))
                dap = A_all[:].rearrange("p (h n) -> p h n", h=2)[
                    :, :, cs0:cs0 + NCW]
                pap = pj[:].rearrange("p (h n) -> p h n", h=2)
                if PRJ_ACT:
                    nc.scalar.copy(dap, pap)
                else:
                    nc.vector.tensor_copy(dap, pap)

            def proj_v(ncc, xt):
                # V projected directly transposed: out[n-pos, dv] tiles
                for sub in (0, 1):
                    nt = 2 * ncc + sub
                    vps = psA.tile([128, 256], f32, tag=f"o{sub}",
                                   name=f"vps_{nt}")
                    for rc8 in range(8):
                        t = 4 + rc8 // 4
                        rc = rc8 % 4
                        lhsT = xt[:, (t * 4 + rc) * NCW + sub * 128:
                                  (t * 4 + rc) * NCW + sub * 128 + 128]
                        nc.tensor.matmul(vps[:], lhsT,
                                         wvp[:, rc8 * 256:rc8 * 256 + 256],
                                         start=(rc8 == 0), stop=(rc8 == 7))
                    dap = v16_all[:].rearrange("p (h n) -> p h n", h=2)[
                        :, :, nt * 128:(nt + 1) * 128]
                    pap = vps[:].rearrange("p (h d) -> p h d", h=2)
                    if V16_DVE:
                        nc.vector.tensor_copy(dap, pap)
                    else:
                        nc.scalar.copy(dap, pap)

            def proj_av(ncc, xt):
                proj_A(ncc, xt)
                proj_v(ncc, xt)

            # ---- attention helpers ----
            def scores_tile(qc, kt, h, bt):
                qs0 = qc * QCW
                ks = slice(h * NK + kt * 128, h * NK + (kt + 1) * 128)
                qsl = slice(h * NQ + qs0, h * NQ + qs0 + QCW)
                sb = psA.tile([128, 1024], f32, tag="sb", bufs=2,
                              name=f"sb_{qc}_{kt}_{h}")
                nc.tensor.matmul(sb[:, 0:512], A_all[:, ks],
                                 q_all[:, qsl], start=True, stop=True)
                nc.tensor.matmul(sb[:, 512:1024], A_all[:, ks],
                                 q2_all[:, qsl], start=True, stop=True)
                # square extraction: sq = sb*sb (re^2 | im^2), f16
                sq = work.tile([128, 1024], f16, tag="sq", bufs=6,
                               name=f"sq_{qc}_{kt}_{h}")
                tile_in_half = (kt % HKT) * 2 + h
                if qc == 0:
                    fset = ACT_FUSED_Q0 if kt < HKT else ACT_FUSED_Q0B
                elif kt < HKT:
                    fset = ACT_FUSED_A
                else:
                    fset = ACT_FUSED_B
                fused = tile_in_half in fset
                if fused:
                    # unary ACT square reads psum once (Square is in every
                    # act table -> no table-load cost)
                    nc.scalar.square(sq[:], sb[:])
                else:
                    # DVE psum->f16 copy, then ONE Pool square over the
                    # whole [128,1024] tile
                    t16 = work.tile([128, 1024], f16, tag="t16", bufs=6,
                                    name=f"t16_{qc}_{kt}_{h}")
                    nc.vector.tensor_copy(t16[:], sb[:])
                    nc.gpsimd.tensor_tensor(sq[:], t16[:], t16[:], ALU.mult)
                # z = re^2 + im^2 -> bt column slot.  The last tiles of
                # each half go to DVE (nearly idle at chunk end) so the
                # final sqrt chunk -- which gates the exps of this chunk
                # and thus the next chunk's AV -- isn't stuck behind the
                # Pool queue.
                zdst = bt[:, kt * 1024 + h * 512:kt * 1024 + h * 512 + 512]
                zdve = (tile_in_half >= 14
                        or cnt["zadd"] % ZADD_MOD < ZADD_DVE_NUM)
                if zdve:
                    nc.vector.tensor_tensor(zdst, sq[:, 0:512],
                                            sq[:, 512:1024], ALU.add)
                else:
                    nc.gpsimd.tensor_tensor(zdst, sq[:, 0:512],
                                            sq[:, 512:1024], ALU.add)
                cnt["zadd"] += 1

            def av_alloc(qc, last=False):
                if last:
                    # final block: both heads in ONE aux-tag psum tile
                    # (aux is free since rowsums run on Pool), so the
                    # final AV does not alias o0/o1 and need not wait
                    # for the previous chunk's normalization reads.
                    opair = psA.tile([128, 1024], f32, tag="aux",
                                     name=f"opair_{qc}")
                    o_ps = [opair[:, 0:512], opair[:, 512:1024]]
                else:
                    o_ps = [psA.tile([128, QCW], f32, tag=f"o{h}",
                                     name=f"o{h}_{qc}") for h in (0, 1)]
                if last:
                    # final block: rowsums accumulate on Pool (idle in
                    # the tail) into f16 sbuf as TWO interleaved chains
                    # (even/odd kt) to halve the serial latency, folded
                    # once by PE at the end.
                    rs = work.tile([128, 2048], f16, tag="rs_acc",
                                   bufs=1, name=f"rsacc_{qc}")
                else:
                    rs = psA.tile([128, 1024], f32, tag="aux",
                                  name=f"auxrs_{qc}")
                return o_ps, rs

            def av_tile(qc, bt, kt, o_ps, rs, last=False):
                for h in (0, 1):
                    vblk = v16_all[:, h * NK + kt * 128:
                                   h * NK + (kt + 1) * 128]
                    pcol = bt[:, kt * 1024 + h * 512:
                              kt * 1024 + h * 512 + 512]
                    nc.tensor.matmul(o_ps[h][:, :], vblk, pcol,
                                     start=(kt == 0), stop=(kt == KT - 1))
                pk = bt[:, kt * 1024:(kt + 1) * 1024]
                if last:
                    # rowsum accumulation on Pool (sbuf only), chain kt%2
                    half = rs[:, (kt % 2) * 1024:(kt % 2) * 1024 + 1024]
                    if kt < 2:
                        nc.gpsimd.tensor_scalar_mul(half, pk, 1.0)
                    else:
                        nc.gpsimd.tensor_tensor(half, half, pk, ALU.add)
                else:
                    # rowsum via PE directly into aux psum row 0
                    # (split 2x512 so each matmul stays within one bank)
                    for cb in (0, 1):
                        nc.tensor.matmul(
                            rs[0:1, cb * 512:(cb + 1) * 512], ones16[:],
                            pk[:, cb * 512:(cb + 1) * 512],
                            start=(kt == 0), stop=(kt == KT - 1))

            def tail_pre(qc, o_ps, rs, last=False):
                qs = slice(qc * QCW, qc * QCW + QCW)
                if last:
                    # fold the two Pool rowsum chains into o0/o1-tag
                    # psum tiles (free after the last chunk's norm read),
                    # then reciprocal + broadcast per column block.
                    folds = [psA.tile([128, 512], f32, tag=f"o{cb}",
                                      name=f"fold_{cb}_{qc}")
                             for cb in (0, 1)]
                    for cb in (0, 1):
                        dst = folds[cb][0:1, :]
                        nc.tensor.matmul(dst, ones16[:],
                                         rs[:, cb * 512:cb * 512 + 512],
                                         start=True, stop=False)
                        nc.tensor.matmul(dst, ones16[:],
                                         rs[:, 1024 + cb * 512:
                                            1024 + cb * 512 + 512],
                                         start=False, stop=True)
                    rcp = work.tile([1, 1024], f32r, tag="rcp",
                                    name=f"rcp_{qc}")
                    for cb in (0, 1):
                        nc.vector.reciprocal(
                            rcp[:, cb * 512:(cb + 1) * 512],
                            folds[cb][0:1, :])
                    bcs = [psA.tile([128, 512], f32, tag=f"o{cb}",
                                    name=f"bcl_{cb}_{qc}")
                           for cb in (0, 1)]
                    bc_sb = work.tile([128, 1024], BCSB_DT, tag="bc_sb",
                                      name=f"bcsb_{qc}")
                    for cb in (0, 1):
                        nc.tensor.matmul(bcs[cb][:, :], ones_row[:],
                                         rcp[:, cb * 512:(cb + 1) * 512],
                                         start=True, stop=True)
                        nc.vector.tensor_copy(
                            bc_sb[:, cb * 512:(cb + 1) * 512],
                            bcs[cb][:, :])
                else:
                    rcp = work.tile([1, 1024], f32r, tag="rcp",
                                    name=f"rcp_{qc}")
                    nc.vector.reciprocal(rcp[:], rs[0:1, :])
                    bc = psA.tile([128, 1024], f32, tag="aux",
                                  name=f"bc_{qc}")
                    for cb in (0, 1):
                        nc.tensor.matmul(bc[:, cb * 512:(cb + 1) * 512],
                                         ones_row[:],
                                         rcp[:, cb * 512:(cb + 1) * 512],
                                         start=True, stop=True)
                    bc_sb = work.tile([128, 1024], BCSB_DT, tag="bc_sb",
                                      name=f"bcsb_{qc}")
                    if BC_ACT:
                        nc.scalar.copy(bc_sb[:], bc[:])
                    else:
                        nc.vector.tensor_copy(bc_sb[:], bc[:])
                # normalize AV output into oT (psum -> f16 sbuf)
                for h in (0, 1):
                    for ri, dest in ((0, oT_re), (1, oT_im)):
                        rows = slice(64 * ri, 64 * ri + 64)
                        nc.vector.scalar_tensor_tensor(
                            dest[64 * h:64 * h + 64, qs],
                            o_ps[h][rows, :], 1.0,
                            bc_sb[rows, h * 512:h * 512 + 512],
                            ALU.mult, ALU.mult)

            def tail_post(qc):
                qs = slice(qc * QCW, qc * QCW + QCW)
                for Rc in range(4):
                    def wob(w):
                        return wop[:, w * 512 + Rc * 128:
                                   w * 512 + Rc * 128 + 128]

                    for ri, (wa, wb_, dst_e) in enumerate(
                            ((0, 2, ore_e), (1, 0, oim_e))):
                        wo = psA.tile([128, 512], f32, tag="sb", bufs=2,
                                      name=f"wo_{Rc}_{ri}_{qc}")
                        nc.tensor.matmul(wo[:], wob(wa), oT_re[:, qs],
                                         start=True, stop=False)
                        nc.tensor.matmul(wo[:], wob(wb_), oT_im[:, qs],
                                         start=False, stop=True)
                        st = work.tile([128, 512], f16, tag="st", bufs=6,
                                       name=f"st_{Rc}_{ri}_{qc}")
                        if qc == QC - 1:
                            act_st = cnt["stc"] % 2 == 0  # parallel tail
                        else:
                            act_st = cnt["stc"] % ST_MOD < ST_ACT_NUM
                        if act_st:
                            nc.scalar.copy(st[:], wo[:])
                        else:
                            nc.vector.tensor_copy(st[:], wo[:])
                        cnt["stc"] += 1
                        nc.sync.dma_start(
                            dst_e[Rc * 128:(Rc + 1) * 128, qs], st[:])

            # ---- main schedule ----
            # Per q-chunk period: [A: AV+rowsum of qc-1 interleaved with
            # scores(qc, half0); sqrt(h0)] [B: tail of qc-1; scores(qc,
            # half1); sqrt(h1); exp(h0); exp(h1)].
            # pre-stream: only the q/A projections of n-chunks 0,1 (all
            # that scores(qc0, kt0..3) need); V projections are deferred
            # into the qc0 stream so PE reaches the first score matmul
            # without waiting for the v-block and wvp DMAs.
            proj_q(0, xt0_pre)
            proj_A(0, xt0_pre)
            proj_q(1, xt1_pre0)
            proj_A(1, xt1_pre0)
            pend = None
            vdefer = []              # (ncc, xt) V-projs deferred to qc1
            vslots_a = (1, 3)        # qc1 half-A slots for V(6), V(7)
            for qc in range(QC):
                bt = work.tile([128, KT * 1024], f16, tag="batch",
                               bufs=2, name=f"bt_{qc}")
                if pend is not None:
                    pqc, pbt = pend
                    o_ps, rs = av_alloc(pqc)

                # AV(pqc) packed into stream slots: delayed enough that
                # the matching exp chunk is ready before PE (in-order
                # queue) reaches the AV matmul, with the late k-tiles
                # doubled up so nothing trails past the stream.  qc1 uses
                # a later map: qc0's exps only start at its very end.
                slots = AV_SLOTS_QC1 if (pend is not None and pqc == 0)                     else AV_SLOTS

                def av_slot(k8):
                    if pend is not None:
                        for kt in slots.get(k8, ()):
                            av_tile(pqc, pbt, kt, o_ps, rs)

                def sqrt_chunk(half, c):
                    # sqrt chunk: ready as soon as its z-adds land;
                    # emitting inline spreads ACT work through the stream
                    # instead of one 7us blob at the half boundary.
                    w = HKT * 1024 // SQRT_CHUNKS
                    cs = slice(half * HKT * 1024 + c * w,
                               half * HKT * 1024 + (c + 1) * w)
                    chain(nc.scalar.activation(bt[:, cs], bt[:, cs],
                                               AF.Sqrt, scale=1.0 / 64.0))

                # A: scores(qc, h0-half k-tiles) + delayed AV(pqc)
                proj_a = {2: 2, 3: 3, 5: 4, 7: 5}   # k8 -> ncc (qc0 only)
                for k8 in range(HKT):
                    if qc == 0:
                        if k8 == 0:
                            proj_v(0, xt0_pre)
                        elif k8 == 1:
                            proj_v(1, xt1_pre0)
                        elif k8 in proj_a:
                            proj_av(proj_a[k8], xt_load(proj_a[k8]))
                    if qc == 1 and k8 in vslots_a and vdefer:
                        proj_v(*vdefer[vslots_a.index(k8)])
                    if qc == 1 and k8 == PJQ_SLOT_A:
                        proj_q_deferred(4)
                    if qc == 2 and k8 == PJQ_SLOT_A:
                        proj_q_deferred(6)
                    av_slot(k8)
                    scores_tile(qc, k8, 0, bt)
                    scores_tile(qc, k8, 1, bt)
                    per = HKT // SQRT_CHUNKS
                    if k8 % per == per - 1:
                        sqrt_chunk(0, k8 // per)
                if ((qc == QC - 1 and QC3_EARLY_EXP)
                        or (EXP_EARLY_HALF and qc != 0)
                        or (EXP_EARLY_QC0 and qc == 0)):
                    # exp the h0-half right after its sqrt chunks (costs
                    # 2 extra table loads, but un-gates the next chunk's
                    # first 8 AV k-tiles)
                    for hh in range(2):
                        cs = slice(hh * 4096, (hh + 1) * 4096)
                        chain(nc.scalar.activation(bt[:, cs], bt[:, cs],
                                                   AF.Exp, bias=eb_exp[:]))
                # B: scores(qc, h1-half) + delayed AV(pqc)
                proj_b = {0: 6, 1: 7}               # k8 -> ncc (qc0 only)
                for k8 in range(HKT):
                    if qc == 0 and k8 in proj_b:
                        ncc_ = proj_b[k8]
                        xt_ = xt_load(ncc_)
                        # xt6/xt7 are the last xt-tag loads, so they stay
                        # alive into qc1 for free: defer their V-projs
                        # (needed only by AV(qc0) k-tiles 12-15 in qc1).
                        if V_DEFER:
                            proj_A(ncc_, xt_)
                            vdefer.append((ncc_, xt_))
                        else:
                            proj_av(ncc_, xt_)
                    if qc == 0 and k8 == 4:
                        proj_q_deferred(2)
                    if qc == 0 and k8 == 5:
                        proj_q_deferred(3)
                    if qc == 1 and k8 == 4:
                        proj_q_deferred(5)
                    if qc == 2 and k8 == 4:
                        proj_q_deferred(7)
                    av_slot(HKT + k8)
                    scores_tile(qc, HKT + k8, 0, bt)
                    scores_tile(qc, HKT + k8, 1, bt)
                    if k8 % per == per - 1:
                        sqrt_chunk(1, k8 // per)
                # exps emitted directly after the sqrt (before the AV
                # trailing tiles / tail) so the scheduler keeps them early
                # in the ACT queue; split so AV can chase chunk by chunk
                if qc != QC - 1:
                    w = KT * 1024 // EXP_SPLIT
                    h0_done = ((EXP_EARLY_HALF and qc != 0)
                               or (EXP_EARLY_QC0 and qc == 0))
                    for hh in range(EXP_SPLIT // 2 if h0_done
                                    else 0, EXP_SPLIT):
                        cs = slice(hh * w, (hh + 1) * w)
                        chain(nc.scalar.activation(bt[:, cs], bt[:, cs],
                                                   AF.Exp, bias=eb_exp[:]))
                else:
                    # split the last exps so the final AV can chase chunk
                    # by chunk; if the early-exp block ran, h0 is done.
                    q0 = 8 if QC3_EARLY_EXP else 0
                    for qtr in range(q0, 16):
                        cs = slice(qtr * 1024, (qtr + 1) * 1024)
                        chain(nc.scalar.activation(bt[:, cs], bt[:, cs],
                                                   AF.Exp, bias=eb_exp[:]))
                if pend is not None:
                    tail_pre(pqc, o_ps, rs)
                    tail_post(pqc)
                pend = (qc, bt)
            # final q-chunk: AV with Pool rowsums, then tail
            pqc, pbt = pend
            o_ps, rs = av_alloc(pqc, last=LAST_POOL_RS)
            for kt in range(KT):
                av_tile(pqc, pbt, kt, o_ps, rs, last=LAST_POOL_RS)
            tail_pre(pqc, o_ps, rs, last=LAST_POOL_RS)
            tail_post(pqc)

    nc.finalize()
    return nc


def _get_nc():
    if "nc" not in _CACHE:
        _CACHE["nc"] = _build_nc()
    return _CACHE["nc"]


def _core_inputs(c, inputs):
    b = c // 4
    h0 = 2 * (c % 4)
    hs = slice(h0 * 64, h0 * 64 + 128)

    xpack = np.empty((NCC, 24, 128, NCW), np.float16)
    for t, name in enumerate(
            ("Q_real", "Q_imag", "K_real", "K_imag", "V_real", "V_imag")):
        xT = np.ascontiguousarray(inputs[name][b].T)          # (512, 2048)
        xpack[:, t * 4:(t + 1) * 4] = (
            xT.reshape(4, 128, NCC, NCW).transpose(2, 0, 1, 3))

    # q and A(k) stationary weights: 8 mats of 4 rc-chunks
    wlist = []
    for kind in ("q", "A"):
        base_r = inputs[{"q": "wq_r", "A": "wk_r"}[kind]]
        base_i = inputs[{"q": "wq_i", "A": "wk_i"}[kind]]
        for hh in (0, 1):
            rows = slice((h0 + hh) * 64, (h0 + hh) * 64 + 64)
            wr, wi_ = base_r[rows], base_i[rows]
            # rows of the projected tensor: [p_r; p_i]
            w1 = np.vstack([wr, wi_])        # x_re weights
            w2 = np.vstack([-wi_, wr])       # x_im weights
            wlist += [w1, w2]
    arr = np.empty((32, 128, 128), np.float16)
    for wi, mat in enumerate(wlist):
        arr[wi * 4:(wi + 1) * 4] = np.ascontiguousarray(mat.T).reshape(
            4, 128, 128)
    wpack = np.ascontiguousarray(arr.transpose(1, 0, 2)).reshape(
        128, 32 * 128)

    # V moving-side (rhs) weights: 8 contract-chunks [128, 256]
    # cols per chunk: h0:[vr|vi] h1:[vr|vi]
    wvr, wvi = inputs["wv_r"], inputs["wv_i"]
    wvarr = np.empty((8, 128, 256), np.float32)
    for rc8 in range(8):
        xi_part = rc8 >= 4
        rc = rc8 % 4
        rs_ = slice(rc * 128, (rc + 1) * 128)
        for hh in (0, 1):
            rows = slice((h0 + hh) * 64, (h0 + hh) * 64 + 64)
            wr_T = wvr[rows].T[rs_]          # (128, 64)
            wi_T = wvi[rows].T[rs_]
            if not xi_part:                  # x_re contribution
                vr_cols, vi_cols = wr_T, wi_T
            else:                            # x_im contribution
                vr_cols, vi_cols = -wi_T, wr_T
            wvarr[rc8, :, hh * 128:hh * 128 + 64] = vr_cols
            wvarr[rc8, :, hh * 128 + 64:hh * 128 + 128] = vi_cols
    wvpack = np.ascontiguousarray(
        wvarr.transpose(1, 0, 2)).reshape(128, 8 * 256).astype(np.float16)

    wo_r_T = np.ascontiguousarray(inputs["wo_r"][:, hs].T)    # (128, 512)
    wo_i_T = np.ascontiguousarray(inputs["wo_i"][:, hs].T)
    wopack = np.concatenate([wo_r_T, wo_i_T, -wo_i_T], axis=1)
    wopack = np.ascontiguousarray(wopack).astype(np.float16)

    return {
        "xpack": xpack,
        "wpack": wpack,
        "wvpack": wvpack,
        "wopack": wopack,
        "onesr": np.ones((1, 128), np.float32),
    }


def kernel(**inputs):
    from concourse.bass_utils import run_bass_kernel_spmd

    nc = _get_nc()
    in_maps = [_core_inputs(c, inputs) for c in range(NCORES)]
    res = run_bass_kernel_spmd(nc, in_maps, list(range(NCORES)))
    # the axon/NRT path very occasionally returns garbage buffers from a
    # transient device fault; retry once if any output is non-finite
    for _ in range(2):
        bad = any(not np.isfinite(
                      res.results[c][k].astype(np.float64)).all()
                  for c in range(NCORES) for k in ("out_re", "out_im"))
        if not bad:
            break
        res = run_bass_kernel_spmd(nc, in_maps, list(range(NCORES)))
    out = np.empty((B, NQ, R, 2), np.float32)
    for b in range(B):
        re = np.zeros((512, NQ), np.float64)
        im = np.zeros((512, NQ), np.float64)
        for c in range(b * 4, b * 4 + 4):
            re += res.results[c]["out_re"].astype(np.float64)
            im += res.results[c]["out_im"].astype(np.float64)
        out[b, :, :, 0] = re.T
        out[b, :, :, 1] = im.T
    return out


# revision 70
# speedup vs baseline: 1.0024x; 1.0024x over previous
"""Complex-valued multi-head attention on 8 Trainium2 NeuronCores.

Sharding: batch(2) x head-pairs(4) -> 8 cores; each core runs one batch
element and 2 heads end-to-end (QKV proj -> complex scores -> |s| softmax
-> AV -> partial W_O), host sums the W_O partials over the 4 cores of each
batch element (tensor-parallel reduce) and transposes to the output layout.

Restructure vs the 257456ns baseline (engine-balance rework, 236107ns
cost-model):
- score extraction: sq = re^2|im^2 either as ONE fused ACT Square
  [128,1024] (psum read, unary -- Square lives in every act table so it
  never forces a table load), or as DVE psum->f16 copy + ONE Pool sbuf
  square [128,1024]; z-adds [128,512] on Pool (last 2 tiles per half on
  DVE so the final sqrt chunk isn't stuck behind the Pool queue).
- V is projected directly in transposed ([k-pos, dv]) layout via
  moving-side weights (wvpack rhs blocks), eliminating the 32 PE
  transposes and the second extraction pass; V(6),V(7) deferred into
  qc1's stream (their xt tiles are the last loads, so they stay live).
- softmax rowsums accumulate on PE (ones matmul into a psum row).
- sqrt runs as 2048-col chunks emitted inline with the score stream;
  exps split x8 and chained after them.  All table-based ACT ops carry
  explicit chain deps (add_dep_helper) so the scheduler cannot
  interleave Sqrt/Exp phases: 11 table loads total (vs 47 unpinned).
- AV(prev qc) is packed into score-stream slots (AV_SLOTS), delayed so
  the in-order PE queue never waits for an exp chunk, late k-tiles
  doubled up so nothing trails the stream.
- q-projections of n-chunks 2..7 deferred out of qc0 (small xtq
  re-DMAs) to shorten the PE-bound qc0; startup DMAs interleaved
  (xt thirds with the weight segment each projection actually needs).
- DMA ring parallelism: weight DMAs and the in-stream xt v-blocks are
  issued from the gpsimd (Pool) DGE ring while the x tensors stream on
  the sync ring, halving the serialized startup-DMA span.
- W_O partials written as f16 (host sums in f64; quantization ~5e-4 rel).
"""
import sys

sys.path.insert(0, "/opt/trn_rl_repo")

import numpy as np

B, NQ, NK, R = 2, 2048, 2048, 512
H, DK, DV = 8, 64, 64
NCORES = 8
NCC = 8          # n-chunks for projection streaming (2048/256)
NCW = 256        # projection n-chunk width
QC = 4           # q-chunks in attention (2048/512)
QCW = 512
KT = 16          # k-tiles (2048/128)
HKT = 8          # k-tiles per half-batch

# engine-mix tuning
# fused-square positions per half: in half A the ACT is busy with the
# previous chunk's exps until ~tile 11, so fused tiles sit late there.
ACT_FUSED_A = (11, 13)
ACT_FUSED_B = (1, 3, 5, 9, 11, 13)
ACT_FUSED_Q0 = range(0, 12)              # qc0 half A (no exps on ACT)
ACT_FUSED_Q0B = range(0, 12)             # qc0 half B (gates sqrt_B_c3)
ZADD_DVE_NUM = 0      # out of ZADD_MOD z-adds go to DVE (rest Pool)
ZADD_MOD = 8
ST_ACT_NUM = 0        # out of ST_MOD W_O output copies go to ACT
ST_MOD = 5
PRJ_ACT = False       # q/A projection copies on ACT
V16_DVE = True        # v16 extraction on DVE
AV_DELAY = 3          # k-tile pairs to delay AV behind the score stream
EXP_SPLIT = 8         # exp batch granularity (ops per q-chunk)
EXP_EARLY_HALF = False  # emit h0-half exps right after sqrt_A chunks
EXP_EARLY_QC0 = False  # qc0 early-half exps: delays sqrt_B, loses
PJQ_SLOT_A = 4        # half-A slot for the deferred q projection
QC3_EARLY_EXP = True  # qc3 h0-half exps between the halves
BC_ACT = False        # bc_sb copy on ACT
PJQ_ACT = False       # deferred-q projection copy on ACT
V_DEFER = True        # defer V(6),V(7) projections into qc1
LAST_POOL_RS = False  # final AV block: rowsums on Pool instead of PE
SQRT_CHUNKS = 8       # sqrt chunks per half-stream
BCSB_F16 = True       # bc_sb (1/rowsum broadcast) in f16
AV_SLOTS = {3: (0,), 4: (1,), 5: (2,), 6: (3, 4),
            7: (5,), 8: (6, 7), 9: (8,), 10: (9, 10),
            11: (11,), 12: (12,), 13: (13,), 14: (14,), 15: (15,)}
AV_SLOTS_QC1 = AV_SLOTS

_CACHE = {}


def _build_nc():
    import concourse.bass as bass
    import concourse.tile as tile
    from concourse.tile import add_dep_helper
    from concourse import bacc, mybir

    f32 = mybir.dt.float32
    f32r = mybir.dt.float32r
    f16 = mybir.dt.float16
    ALU = mybir.AluOpType
    AF = mybir.ActivationFunctionType
    BCSB_DT = f16 if BCSB_F16 else f32

    nc = bacc.Bacc("TRN2", target_bir_lowering=False, debug=False,
                   num_devices=NCORES)

    xpack_e = nc.dram_tensor("xpack", [NCC, 24, 128, NCW], f16,
                             kind="ExternalInput")
    wpack_e = nc.dram_tensor("wpack", [128, 32 * 128], f16,
                             kind="ExternalInput")
    wvpack_e = nc.dram_tensor("wvpack", [128, 8 * 256], f16,
                              kind="ExternalInput")
    wopack_e = nc.dram_tensor("wopack", [128, 3 * 512], f16,
                              kind="ExternalInput")
    onesr_e = nc.dram_tensor("onesr", [1, 128], f32r, kind="ExternalInput")
    ore_e = nc.dram_tensor("out_re", [512, NQ], f16, kind="ExternalOutput")
    oim_e = nc.dram_tensor("out_im", [512, NQ], f16, kind="ExternalOutput")

    with tile.TileContext(nc) as tc:
      with nc.allow_low_precision(reason="fp16 softmax path"):
        with tc.tile_pool(name="pers", bufs=1) as pers, \
             tc.tile_pool(name="work", bufs=2) as work, \
             tc.tile_pool(name="psA", bufs=1, space="PSUM") as psA:

            # startup DMA order: interleave xt0 thirds with the weight
            # segments they unblock, so the first projection matmul can
            # start after ~1/3 of xt0 + wp seg0 instead of the full set.
            wp = pers.tile([128, 32 * 128], f16, tag="wp")
            wvp = pers.tile([128, 8 * 256], f16, tag="wvp")
            xt0_pre = work.tile([128, 24 * NCW], f16, tag="xt",
                                name="xt_0")

            def xt_part(xt, ncc, t0, t1):
                nc.sync.dma_start(
                    xt[:, t0 * 4 * NCW:t1 * 4 * NCW].rearrange(
                        "p (b f) -> p b f", f=NCW),
                    xpack_e[ncc, t0 * 4:t1 * 4].rearrange("b p f -> p b f"))

            xt1_pre0 = work.tile([128, 24 * NCW], f16, tag="xt",
                                 name="xt_1")
            xt_part(xt0_pre, 0, 0, 2)                       # q blocks
            nc.gpsimd.dma_start(wp[:, 0:2048], wpack_e[:, 0:2048])
            xt_part(xt0_pre, 0, 2, 4)                       # k blocks
            nc.gpsimd.dma_start(wp[:, 2048:4096], wpack_e[:, 2048:4096])
            xt_part(xt1_pre0, 1, 0, 2)
            xt_part(xt1_pre0, 1, 2, 4)
            nc.gpsimd.dma_start(wvp[:], wvpack_e[:])
            xt_part(xt0_pre, 0, 4, 6)                       # v blocks
            xt_part(xt1_pre0, 1, 4, 6)
            wop = pers.tile([128, 3 * 512], f16, tag="wop")
            nc.gpsimd.dma_start(wop[:], wopack_e[:])
            ones_row = pers.tile([1, 128], f32r, tag="ones_row")
            nc.gpsimd.dma_start(ones_row[:], onesr_e[:])
            ones16 = pers.tile([128, 1], f16, tag="ones16")
            nc.vector.memset(ones16[:], 1.0)
            eb_exp = pers.tile([128, 1], f32, tag="eb_exp")
            nc.vector.memset(eb_exp[:], -1.5)          # exp(mag - 1.5)

            # ---- projection destinations (h-major: cols h*2048 + n) ----
            q_all = pers.tile([128, 2 * NQ], f16, tag="q_all")
            q2_all = pers.tile([128, 2 * NQ], f16, tag="q2_all")
            A_all = pers.tile([128, 2 * NK], f16, tag="A_all")
            v16_all = pers.tile([128, 2 * NK], f16, tag="v16_all")
            oT_re = pers.tile([128, NQ], f16, tag="oT_re")
            oT_im = pers.tile([128, NQ], f16, tag="oT_im")

            grp_dest = [q_all, A_all]

            cnt = {"zadd": 0, "stc": 0}
            # ACT table-phase chaining: Sqrt and Exp live in different act
            # function tables (Square/Copy are in every table).  Without
            # explicit edges the scheduler interleaves sqrt chunks with
            # exp chunks and pays 1283ns per table switch.  Chaining all
            # table-based ACT ops in emission order keeps each table phase
            # contiguous (emission order already groups them).
            phase = {"last": None}

            def chain(ins):
                if phase["last"] is not None:
                    add_dep_helper(ins.ins, phase["last"].ins,
                                   reason="act table phase order")
                phase["last"] = ins
                return ins

            def xt_load(ncc):
                # k blocks first (A projection is the critical path) on
                # the sync ring, v blocks in parallel on the Pool ring.
                xt = work.tile([128, 24 * NCW], f16, tag="xt",
                               name=f"xt_{ncc}")
                xt_part(xt, ncc, 2, 4)
                nc.gpsimd.dma_start(
                    xt[:, 16 * NCW:24 * NCW].rearrange(
                        "p (b f) -> p b f", f=NCW),
                    xpack_e[ncc, 16:24].rearrange("b p f -> p b f"))
                return xt

            def wblk(w, rc):
                return wp[:, (w * 4 + rc) * 128:(w * 4 + rc + 1) * 128]

            def proj_q(ncc, xt, xoff=0):
                """q projection + q2 for n-chunk ncc; xoff=None marks the
                deferred path (q copy goes to ACT instead of DVE)."""
                def xblk(t, rc):
                    b = ((xoff or 0) + t * 4 + rc)
                    return xt[:, b * NCW:(b + 1) * NCW]

                cs0 = ncc * NCW
                pj = psA.tile([128, 512], f32, tag="sb", bufs=2,
                              name=f"pjq_{ncc}")
                for sub in range(2):
                    dst = pj[:, sub * 256:(sub + 1) * 256]
                    for rc in range(4):
                        nc.tensor.matmul(dst, wblk(2 * sub, rc),
                                         xblk(0, rc),
                                         start=(rc == 0), stop=False)
                    for rc in range(4):
                        nc.tensor.matmul(dst, wblk(2 * sub + 1, rc),
                                         xblk(1, rc),
                                         start=False, stop=(rc == 3))
                dap = q_all[:].rearrange("p (h n) -> p h n", h=2)[
                    :, :, cs0:cs0 + NCW]
                pap = pj[:].rearrange("p (h n) -> p h n", h=2)
                if PRJ_ACT or (xoff is None and PJQ_ACT):
                    nc.scalar.copy(dap, pap)
                else:
                    nc.vector.tensor_copy(dap, pap)
                # q2 = [q_i; -q_r] for this chunk (both heads) on DVE
                q2ap_t = q2_all[0:64].rearrange("p (h n) -> p h n", h=2)[
                    :, :, cs0:cs0 + NCW]
                qap_b = q_all[64:128].rearrange("p (h n) -> p h n", h=2)[
                    :, :, cs0:cs0 + NCW]
                nc.vector.tensor_scalar_mul(q2ap_t, qap_b, 1.0)
                q2ap_b = q2_all[64:128].rearrange("p (h n) -> p h n", h=2)[
                    :, :, cs0:cs0 + NCW]
                qap_t = q_all[0:64].rearrange("p (h n) -> p h n", h=2)[
                    :, :, cs0:cs0 + NCW]
                nc.vector.tensor_scalar_mul(q2ap_b, qap_t, -1.0)

            def proj_q_deferred(ncc):
                # late q projection: re-DMA just the q blocks of this
                # n-chunk (xt tiles from qc0 are long recycled)
                xtq = work.tile([128, 8 * NCW], f16, tag="xtq",
                                name=f"xtq_{ncc}")
                nc.sync.dma_start(
                    xtq[:].rearrange("p (b f) -> p b f", f=NCW),
                    xpack_e[ncc, 0:8].rearrange("b p f -> p b f"))
                proj_q(ncc, xtq, xoff=None)

            def proj_A(ncc, xt):
                def xblk(t, rc):
                    return xt[:, (t * 4 + rc) * NCW:(t * 4 + rc + 1) * NCW]

                cs0 = ncc * NCW
                # A (k) projection: stationary weights, [out, n]
                pj = psA.tile([128, 512], f32, tag="sb", bufs=2,
                              name=f"pjA_{ncc}")
                for sub in range(2):
                    s = 2 + sub
                    dst = pj[:, sub * 256:(sub + 1) * 256]
                    for rc in range(4):
                        nc.tensor.matmul(dst, wblk(2 * s, rc),
                                         xblk(2, rc),
                                         start=(rc == 0), stop=False)
                    for rc in range(4):
                        nc.tensor.matmul(dst, wblk(2 * s + 1, rc),
                                         xblk(3, rc),
                                         start=False, stop=(rc == 3))
                dap = A_all[:].rearrange("p (h n) -> p h n", h=2)[
                    :, :, cs0:cs0 + NCW]
                pap = pj[:].rearrange("p (h n) -> p h n", h=2)
                if PRJ_ACT:
                    nc.scalar.copy(dap, pap)
                else:
                    nc.vector.tensor_copy(dap, pap)

            def proj_v(ncc, xt):
                # V projected directly transposed: out[n-pos, dv] tiles
                for sub in (0, 1):
                    nt = 2 * ncc + sub
                    vps = psA.tile([128, 256], f32, tag=f"o{sub}",
                                   name=f"vps_{nt}")
                    for rc8 in range(8):
                        t = 4 + rc8 // 4
                        rc = rc8 % 4
                        lhsT = xt[:, (t * 4 + rc) * NCW + sub * 128:
                                  (t * 4 + rc) * NCW + sub * 128 + 128]
                        nc.tensor.matmul(vps[:], lhsT,
                                         wvp[:, rc8 * 256:rc8 * 256 + 256],
                                         start=(rc8 == 0), stop=(rc8 == 7))
                    dap = v16_all[:].rearrange("p (h n) -> p h n", h=2)[
                        :, :, nt * 128:(nt + 1) * 128]
                    pap = vps[:].rearrange("p (h d) -> p h d", h=2)
                    if V16_DVE:
                        nc.vector.tensor_copy(dap, pap)
                    else:
                        nc.scalar.copy(dap, pap)

            def proj_av(ncc, xt):
                proj_A(ncc, xt)
                proj_v(ncc, xt)

            # ---- attention helpers ----
            def scores_tile(qc, kt, h, bt):
                qs0 = qc * QCW
                ks = slice(h * NK + kt * 128, h * NK + (kt + 1) * 128)
                qsl = slice(h * NQ + qs0, h * NQ + qs0 + QCW)
                sb = psA.tile([128, 1024], f32, tag="sb", bufs=2,
                              name=f"sb_{qc}_{kt}_{h}")
                nc.tensor.matmul(sb[:, 0:512], A_all[:, ks],
                                 q_all[:, qsl], start=True, stop=True)
                nc.tensor.matmul(sb[:, 512:1024], A_all[:, ks],
                                 q2_all[:, qsl], start=True, stop=True)
                # square extraction: sq = sb*sb (re^2 | im^2), f16
                sq = work.tile([128, 1024], f16, tag="sq", bufs=6,
                               name=f"sq_{qc}_{kt}_{h}")
                tile_in_half = (kt % HKT) * 2 + h
                if qc == 0:
                    fset = ACT_FUSED_Q0 if kt < HKT else ACT_FUSED_Q0B
                elif kt < HKT:
                    fset = ACT_FUSED_A
                else:
                    fset = ACT_FUSED_B
                fused = tile_in_half in fset
                if fused:
                    # unary ACT square reads psum once (Square is in every
                    # act table -> no table-load cost)
                    nc.scalar.square(sq[:], sb[:])
                else:
                    # DVE psum->f16 copy, then ONE Pool square over the
                    # whole [128,1024] tile
                    t16 = work.tile([128, 1024], f16, tag="t16", bufs=6,
                                    name=f"t16_{qc}_{kt}_{h}")
                    nc.vector.tensor_copy(t16[:], sb[:])
                    nc.gpsimd.tensor_tensor(sq[:], t16[:], t16[:], ALU.mult)
                # z = re^2 + im^2 -> bt column slot.  The last tiles of
                # each half go to DVE (nearly idle at chunk end) so the
                # final sqrt chunk -- which gates the exps of this chunk
                # and thus the next chunk's AV -- isn't stuck behind the
                # Pool queue.
                zdst = bt[:, kt * 1024 + h * 512:kt * 1024 + h * 512 + 512]
                zdve = (tile_in_half >= 14
                        or cnt["zadd"] % ZADD_MOD < ZADD_DVE_NUM)
                if zdve:
                    nc.vector.tensor_tensor(zdst, sq[:, 0:512],
                                            sq[:, 512:1024], ALU.add)
                else:
                    nc.gpsimd.tensor_tensor(zdst, sq[:, 0:512],
                                            sq[:, 512:1024], ALU.add)
                cnt["zadd"] += 1

            def av_alloc(qc, last=False):
                if last:
                    # final block: both heads in ONE aux-tag psum tile
                    # (aux is free since rowsums run on Pool), so the
                    # final AV does not alias o0/o1 and need not wait
                    # for the previous chunk's normalization reads.
                    opair = psA.tile([128, 1024], f32, tag="aux",
                                     name=f"opair_{qc}")
                    o_ps = [opair[:, 0:512], opair[:, 512:1024]]
                else:
                    o_ps = [psA.tile([128, QCW], f32, tag=f"o{h}",
                                     name=f"o{h}_{qc}") for h in (0, 1)]
                if last:
                    # final block: rowsums accumulate on Pool (idle in
                    # the tail) into f16 sbuf as TWO interleaved chains
                    # (even/odd kt) to halve the serial latency, folded
                    # once by PE at the end.
                    rs = work.tile([128, 2048], f16, tag="rs_acc",
                                   bufs=1, name=f"rsacc_{qc}")
                else:
                    rs = psA.tile([128, 1024], f32, tag="aux",
                                  name=f"auxrs_{qc}")
                return o_ps, rs

            def av_tile(qc, bt, kt, o_ps, rs, last=False):
                for h in (0, 1):
                    vblk = v16_all[:, h * NK + kt * 128:
                                   h * NK + (kt + 1) * 128]
                    pcol = bt[:, kt * 1024 + h * 512:
                              kt * 1024 + h * 512 + 512]
                    nc.tensor.matmul(o_ps[h][:, :], vblk, pcol,
                                     start=(kt == 0), stop=(kt == KT - 1))
                pk = bt[:, kt * 1024:(kt + 1) * 1024]
                if last:
                    # rowsum accumulation on Pool (sbuf only), chain kt%2
                    half = rs[:, (kt % 2) * 1024:(kt % 2) * 1024 + 1024]
                    if kt < 2:
                        nc.gpsimd.tensor_scalar_mul(half, pk, 1.0)
                    else:
                        nc.gpsimd.tensor_tensor(half, half, pk, ALU.add)
                else:
                    # rowsum via PE directly into aux psum row 0
                    # (split 2x512 so each matmul stays within one bank)
                    for cb in (0, 1):
                        nc.tensor.matmul(
                            rs[0:1, cb * 512:(cb + 1) * 512], ones16[:],
                            pk[:, cb * 512:(cb + 1) * 512],
                            start=(kt == 0), stop=(kt == KT - 1))

            def tail_pre(qc, o_ps, rs, last=False):
                qs = slice(qc * QCW, qc * QCW + QCW)
                if last:
                    # fold the two Pool rowsum chains into o0/o1-tag
                    # psum tiles (free after the last chunk's norm read),
                    # then reciprocal + broadcast per column block.
                    folds = [psA.tile([128, 512], f32, tag=f"o{cb}",
                                      name=f"fold_{cb}_{qc}")
                             for cb in (0, 1)]
                    for cb in (0, 1):
                        dst = folds[cb][0:1, :]
                        nc.tensor.matmul(dst, ones16[:],
                                         rs[:, cb * 512:cb * 512 + 512],
                                         start=True, stop=False)
                        nc.tensor.matmul(dst, ones16[:],
                                         rs[:, 1024 + cb * 512:
                                            1024 + cb * 512 + 512],
                                         start=False, stop=True)
                    rcp = work.tile([1, 1024], f32r, tag="rcp",
                                    name=f"rcp_{qc}")
                    for cb in (0, 1):
                        nc.vector.reciprocal(
                            rcp[:, cb * 512:(cb + 1) * 512],
                            folds[cb][0:1, :])
                    bcs = [psA.tile([128, 512], f32, tag=f"o{cb}",
                                    name=f"bcl_{cb}_{qc}")
                           for cb in (0, 1)]
                    bc_sb = work.tile([128, 1024], BCSB_DT, tag="bc_sb",
                                      name=f"bcsb_{qc}")
                    for cb in (0, 1):
                        nc.tensor.matmul(bcs[cb][:, :], ones_row[:],
                                         rcp[:, cb * 512:(cb + 1) * 512],
                                         start=True, stop=True)
                        nc.vector.tensor_copy(
                            bc_sb[:, cb * 512:(cb + 1) * 512],
                            bcs[cb][:, :])
                else:
                    rcp = work.tile([1, 1024], f32r, tag="rcp",
                                    name=f"rcp_{qc}")
                    nc.vector.reciprocal(rcp[:], rs[0:1, :])
                    bc = psA.tile([128, 1024], f32, tag="aux",
                                  name=f"bc_{qc}")
                    for cb in (0, 1):
                        nc.tensor.matmul(bc[:, cb * 512:(cb + 1) * 512],
                                         ones_row[:],
                                         rcp[:, cb * 512:(cb + 1) * 512],
                                         start=True, stop=True)
                    bc_sb = work.tile([128, 1024], BCSB_DT, tag="bc_sb",
                                      name=f"bcsb_{qc}")
                    if BC_ACT:
                        nc.scalar.copy(bc_sb[:], bc[:])
                    else:
                        nc.vector.tensor_copy(bc_sb[:], bc[:])
                # normalize AV output into oT (psum -> f16 sbuf)
                for h in (0, 1):
                    for ri, dest in ((0, oT_re), (1, oT_im)):
                        rows = slice(64 * ri, 64 * ri + 64)
                        nc.vector.scalar_tensor_tensor(
                            dest[64 * h:64 * h + 64, qs],
                            o_ps[h][rows, :], 1.0,
                            bc_sb[rows, h * 512:h * 512 + 512],
                            ALU.mult, ALU.mult)

            def tail_post(qc):
                qs = slice(qc * QCW, qc * QCW + QCW)
                for Rc in range(4):
                    def wob(w):
                        return wop[:, w * 512 + Rc * 128:
                                   w * 512 + Rc * 128 + 128]

                    for ri, (wa, wb_, dst_e) in enumerate(
                            ((0, 2, ore_e), (1, 0, oim_e))):
                        wo = psA.tile([128, 512], f32, tag="sb", bufs=2,
                                      name=f"wo_{Rc}_{ri}_{qc}")
                        nc.tensor.matmul(wo[:], wob(wa), oT_re[:, qs],
                                         start=True, stop=False)
                        nc.tensor.matmul(wo[:], wob(wb_), oT_im[:, qs],
                                         start=False, stop=True)
                        st = work.tile([128, 512], f16, tag="st", bufs=6,
                                       name=f"st_{Rc}_{ri}_{qc}")
                        if qc == QC - 1:
                            act_st = cnt["stc"] % 2 == 0  # parallel tail
                        else:
                            act_st = cnt["stc"] % ST_MOD < ST_ACT_NUM
                        if act_st:
                            nc.scalar.copy(st[:], wo[:])
                        else:
                            nc.vector.tensor_copy(st[:], wo[:])
                        cnt["stc"] += 1
                        nc.sync.dma_start(
                            dst_e[Rc * 128:(Rc + 1) * 128, qs], st[:])

            # ---- main schedule ----
            # Per q-chunk period: [A: AV+rowsum of qc-1 interleaved with
            # scores(qc, half0); sqrt(h0)] [B: tail of qc-1; scores(qc,
            # half1); sqrt(h1); exp(h0); exp(h1)].
            # pre-stream: only the q/A projections of n-chunks 0,1 (all
            # that scores(qc0, kt0..3) need); V projections are deferred
            # into the qc0 stream so PE reaches the first score matmul
            # without waiting for the v-block and wvp DMAs.
            proj_q(0, xt0_pre)
            proj_A(0, xt0_pre)
            proj_q(1, xt1_pre0)
            proj_A(1, xt1_pre0)
            pend = None
            vdefer = []              # (ncc, xt) V-projs deferred to qc1
            vslots_a = (1, 3)        # qc1 half-A slots for V(6), V(7)
            for qc in range(QC):
                bt = work.tile([128, KT * 1024], f16, tag="batch",
                               bufs=2, name=f"bt_{qc}")
                if pend is not None:
                    pqc, pbt = pend
                    o_ps, rs = av_alloc(pqc)

                # AV(pqc) packed into stream slots: delayed enough that
                # the matching exp chunk is ready before PE (in-order
                # queue) reaches the AV matmul, with the late k-tiles
                # doubled up so nothing trails past the stream.  qc1 uses
                # a later map: qc0's exps only start at its very end.
                slots = AV_SLOTS_QC1 if (pend is not None and pqc == 0)                     else AV_SLOTS

                def av_slot(k8):
                    if pend is not None:
                        for kt in slots.get(k8, ()):
                            av_tile(pqc, pbt, kt, o_ps, rs)

                def sqrt_chunk(half, c):
                    # sqrt chunk: ready as soon as its z-adds land;
                    # emitting inline spreads ACT work through the stream
                    # instead of one 7us blob at the half boundary.
                    w = HKT * 1024 // SQRT_CHUNKS
                    cs = slice(half * HKT * 1024 + c * w,
                               half * HKT * 1024 + (c + 1) * w)
                    chain(nc.scalar.activation(bt[:, cs], bt[:, cs],
                                               AF.Sqrt, scale=1.0 / 64.0))

                # A: scores(qc, h0-half k-tiles) + delayed AV(pqc)
                proj_a = {2: 2, 3: 3, 5: 4, 7: 5}   # k8 -> ncc (qc0 only)
                for k8 in range(HKT):
                    if qc == 0:
                        if k8 == 0:
                            proj_v(0, xt0_pre)
                        elif k8 == 1:
                            proj_v(1, xt1_pre0)
                        elif k8 in proj_a:
                            proj_av(proj_a[k8], xt_load(proj_a[k8]))
                    if qc == 1 and k8 in vslots_a and vdefer:
                        proj_v(*vdefer[vslots_a.index(k8)])
                    if qc == 1 and k8 == PJQ_SLOT_A:
                        proj_q_deferred(4)
                    if qc == 2 and k8 == PJQ_SLOT_A:
                        proj_q_deferred(6)
                    av_slot(k8)
                    scores_tile(qc, k8, 0, bt)
                    scores_tile(qc, k8, 1, bt)
                    per = HKT // SQRT_CHUNKS
                    if k8 % per == per - 1:
                        sqrt_chunk(0, k8 // per)
                if ((qc == QC - 1 and QC3_EARLY_EXP)
                        or (EXP_EARLY_HALF and qc != 0)
                        or (EXP_EARLY_QC0 and qc == 0)):
                    # exp the h0-half right after its sqrt chunks (costs
                    # 2 extra table loads, but un-gates the next chunk's
                    # first 8 AV k-tiles)
                    for hh in range(2):
                        cs = slice(hh * 4096, (hh + 1) * 4096)
                        chain(nc.scalar.activation(bt[:, cs], bt[:, cs],
                                                   AF.Exp, bias=eb_exp[:]))
                # B: scores(qc, h1-half) + delayed AV(pqc)
                proj_b = {0: 6, 1: 7}               # k8 -> ncc (qc0 only)
                for k8 in range(HKT):
                    if qc == 0 and k8 in proj_b:
                        ncc_ = proj_b[k8]
                        xt_ = xt_load(ncc_)
                        # xt6/xt7 are the last xt-tag loads, so they stay
                        # alive into qc1 for free: defer their V-projs
                        # (needed only by AV(qc0) k-tiles 12-15 in qc1).
                        if V_DEFER:
                            proj_A(ncc_, xt_)
                            vdefer.append((ncc_, xt_))
                        else:
                            proj_av(ncc_, xt_)
                    if qc == 0 and k8 == 4:
                        proj_q_deferred(2)
                    if qc == 0 and k8 == 5:
                        proj_q_deferred(3)
                    if qc == 1 and k8 == 4:
                        proj_q_deferred(5)
                    if qc == 2 and k8 == 4:
                        proj_q_deferred(7)
                    av_slot(HKT + k8)
                    scores_tile(qc, HKT + k8, 0, bt)
                    scores_tile(qc, HKT + k8, 1, bt)
                    if k8 % per == per - 1:
                        sqrt_chunk(1, k8 // per)
                # exps emitted directly after the sqrt (before the AV
                # trailing tiles / tail) so the scheduler keeps them early
                # in the ACT queue; split so AV can chase chunk by chunk
                if qc != QC - 1:
                    w = KT * 1024 // EXP_SPLIT
                    h0_done = ((EXP_EARLY_HALF and qc != 0)
                               or (EXP_EARLY_QC0 and qc == 0))
                    for hh in range(EXP_SPLIT // 2 if h0_done
                                    else 0, EXP_SPLIT):
                        cs = slice(hh * w, (hh + 1) * w)
                        chain(nc.scalar.activation(bt[:, cs], bt[:, cs],
                                                   AF.Exp, bias=eb_exp[:]))
                else:
                    # split the last exps so the final AV can chase chunk
                    # by chunk; if the early-exp block ran, h0 is done.
                    q0 = 8 if QC3_EARLY_EXP else 0
                    for qtr in range(q0, 16):
                        cs = slice(qtr * 1024, (qtr + 1) * 1024)
                        chain(nc.scalar.activation(bt[:, cs], bt[:, cs],
                                                   AF.Exp, bias=eb_exp[:]))
                if pend is not None:
                    tail_pre(pqc, o_ps, rs)
                    tail_post(pqc)
                pend = (qc, bt)
            # final q-chunk: AV with Pool rowsums, then tail
            pqc, pbt = pend
            o_ps, rs = av_alloc(pqc, last=LAST_POOL_RS)
            for kt in range(KT):
                av_tile(pqc, pbt, kt, o_ps, rs, last=LAST_POOL_RS)
            tail_pre(pqc, o_ps, rs, last=LAST_POOL_RS)
            tail_post(pqc)

    nc.finalize()
    return nc


def _get_nc():
    if "nc" not in _CACHE:
        _CACHE["nc"] = _build_nc()
    return _CACHE["nc"]


def _core_inputs(c, inputs):
    b = c // 4
    h0 = 2 * (c % 4)
    hs = slice(h0 * 64, h0 * 64 + 128)

    xpack = np.empty((NCC, 24, 128, NCW), np.float16)
    for t, name in enumerate(
            ("Q_real", "Q_imag", "K_real", "K_imag", "V_real", "V_imag")):
        xT = np.ascontiguousarray(inputs[name][b].T)          # (512, 2048)
        xpack[:, t * 4:(t + 1) * 4] = (
            xT.reshape(4, 128, NCC, NCW).transpose(2, 0, 1, 3))

    # q and A(k) stationary weights: 8 mats of 4 rc-chunks
    wlist = []
    for kind in ("q", "A"):
        base_r = inputs[{"q": "wq_r", "A": "wk_r"}[kind]]
        base_i = inputs[{"q": "wq_i", "A": "wk_i"}[kind]]
        for hh in (0, 1):
            rows = slice((h0 + hh) * 64, (h0 + hh) * 64 + 64)
            wr, wi_ = base_r[rows], base_i[rows]
            # rows of the projected tensor: [p_r; p_i]
            w1 = np.vstack([wr, wi_])        # x_re weights
            w2 = np.vstack([-wi_, wr])       # x_im weights
            wlist += [w1, w2]
    arr = np.empty((32, 128, 128), np.float16)
    for wi, mat in enumerate(wlist):
        arr[wi * 4:(wi + 1) * 4] = np.ascontiguousarray(mat.T).reshape(
            4, 128, 128)
    wpack = np.ascontiguousarray(arr.transpose(1, 0, 2)).reshape(
        128, 32 * 128)

    # V moving-side (rhs) weights: 8 contract-chunks [128, 256]
    # cols per chunk: h0:[vr|vi] h1:[vr|vi]
    wvr, wvi = inputs["wv_r"], inputs["wv_i"]
    wvarr = np.empty((8, 128, 256), np.float32)
    for rc8 in range(8):
        xi_part = rc8 >= 4
        rc = rc8 % 4
        rs_ = slice(rc * 128, (rc + 1) * 128)
        for hh in (0, 1):
            rows = slice((h0 + hh) * 64, (h0 + hh) * 64 + 64)
            wr_T = wvr[rows].T[rs_]          # (128, 64)
            wi_T = wvi[rows].T[rs_]
            if not xi_part:                  # x_re contribution
                vr_cols, vi_cols = wr_T, wi_T
            else:                            # x_im contribution
                vr_cols, vi_cols = -wi_T, wr_T
            wvarr[rc8, :, hh * 128:hh * 128 + 64] = vr_cols
            wvarr[rc8, :, hh * 128 + 64:hh * 128 + 128] = vi_cols
    wvpack = np.ascontiguousarray(
        wvarr.transpose(1, 0, 2)).reshape(128, 8 * 256).astype(np.float16)

    wo_r_T = np.ascontiguousarray(inputs["wo_r"][:, hs].T)    # (128, 512)
    wo_i_T = np.ascontiguousarray(inputs["wo_i"][:, hs].T)
    wopack = np.concatenate([wo_r_T, wo_i_T, -wo_i_T], axis=1)
    wopack = np.ascontiguousarray(wopack).astype(np.float16)

    return {
        "xpack": xpack,
        "wpack": wpack,
        "wvpack": wvpack,
        "wopack": wopack,
        "onesr": np.ones((1, 128), np.float32),
    }


def kernel(**inputs):
    from concourse.bass_utils import run_bass_kernel_spmd

    nc = _get_nc()
    in_maps = [_core_inputs(c, inputs) for c in range(NCORES)]
    res = run_bass_kernel_spmd(nc, in_maps, list(range(NCORES)))
    # the axon/NRT path very occasionally returns garbage buffers from a
    # transient device fault; retry once if any output is non-finite
    for _ in range(2):
        bad = any(not np.isfinite(
                      res.results[c][k].astype(np.float64)).all()
                  for c in range(NCORES) for k in ("out_re", "out_im"))
        if not bad:
            break
        res = run_bass_kernel_spmd(nc, in_maps, list(range(NCORES)))
    out = np.empty((B, NQ, R, 2), np.float32)
    for b in range(B):
        re = np.zeros((512, NQ), np.float64)
        im = np.zeros((512, NQ), np.float64)
        for c in range(b * 4, b * 4 + 4):
            re += res.results[c]["out_re"].astype(np.float64)
            im += res.results[c]["out_im"].astype(np.float64)
        out[b, :, :, 0] = re.T
        out[b, :, :, 1] = im.T
    return out
